# revision 14
# baseline (speedup 1.0000x reference)
"""ESA molecule classifier on 8 Trainium2 NeuronCores (pure data parallel).

Design (per core, 4 graphs):
  - Activations kept feature-major (E^T: [d, L] with d on partitions).
  - Attention: scores computed transposed (s^T [m, l]) with 4-head row-tiled
    matmuls (K=dh=32 -> tile_position=(32j, 0)); exp on ScalarE without max
    subtraction (scores empirically bounded |s| < 8); softmax denominator via
    col-tiled ones-matmuls sharing the attv 128x32 mode; reciprocal via
    exp(-ln(x)) on ScalarE (same ACT table set as the softmax exp).
  - LayerNorm over features (partition dim) via ones-matmul column sums,
    batched over the 4 graphs with col-tiled M=1 matmuls (stats for graph g
    land on psum partition 32g), rstd = exp(-0.5 ln(.)), then (a, c) rows are
    broadcast back with K=1 row-tiled matmuls that also fold in gamma/beta.
  - Host-side algebra: node-MLP l2 and edge-MLP l2 are fused into the eset
    l1 weights (gather commutes with the projection), so the edge-endpoint
    gather becomes a one-hot selection matmul of a [128-node, 512] tile.
"""

import math

import numpy as np
import ml_dtypes

import concourse.bass as bass
import concourse.mybir as mybir
import concourse.tile as tile
from concourse import bacc, bass_utils
from concourse.alu_op_type import AluOpType

FP = mybir.dt.float32
BF = mybir.dt.bfloat16
AF = mybir.ActivationFunctionType

H = 8
HID = 256
NI = 4
L = 512          # edges per graph (attention length)
NN = 128         # nodes per graph
GPC = 4          # graphs per core
NC = 8           # cores
EPS = 1e-5

_CACHE = {}


# --------------------------------------------------------------------------
# host-side weight prep
# --------------------------------------------------------------------------

def _np(x):
    return np.asarray(x, dtype=np.float32)


def _pack(w, nchunk):
    # [nchunk*128, X] -> [128, nchunk*X] with K-chunk k at [:, k*X:(k+1)*X]
    w = _np(w)
    k, x = w.shape
    assert k == nchunk * 128
    return np.concatenate([w[i * 128:(i + 1) * 128] for i in range(nchunk)], axis=1)


def _bf(w):
    return np.asarray(w, dtype=ml_dtypes.bfloat16)


def prep_weights(params):
    p = params
    out = {}

    def lin(d):
        return _np(d['w']), _np(d['b'])

    wn1, bn1 = lin(p['node']['l1'])
    wn2, bn2 = lin(p['node']['l2'])
    we1, be1 = lin(p['edge']['l1'])
    we2, be2 = lin(p['edge']['l2'])
    ws1, bs1 = lin(p['eset']['l1'])
    ws2, bs2 = lin(p['eset']['l2'])

    w1_hs, w1_hd, w1_e = ws1[:HID], ws1[HID:2 * HID], ws1[2 * HID:]
    out['wn1'] = wn1                                   # [32, 256]
    out['wfused'] = _pack(wn2 @ np.concatenate([w1_hs, w1_hd], axis=1), 2)  # [128,1024]
    out['we1'] = we1                                   # [16, 256]
    out['wef'] = _pack(we2 @ w1_e, 2)                  # [128, 512]
    out['w2e'] = _pack(ws2, 2)                         # [128, 512]
    b1p = bs1 + bn2 @ w1_hs + bn2 @ w1_hd + be2 @ w1_e

    bias_cols = []                                     # list of [128] f32

    def add_bias(vec):
        vec = _np(vec)
        idx = len(bias_cols)
        assert vec.shape == (HID,)
        bias_cols.append(vec[:128])
        bias_cols.append(vec[128:])
        return idx

    out['i_bn1'] = add_bias(bn1)
    out['i_be1'] = add_bias(be1)
    out['i_b1p'] = add_bias(b1p)
    out['i_b2e'] = add_bias(bs2)

    ln_blocks = []                                     # per LN: [4, 128] rows

    def add_ln(g, b):
        ln_blocks.append(np.stack([_np(g)[:128], _np(g)[128:],
                                   _np(b)[:128], _np(b)[128:]]))

    for i, sp in enumerate(list(p['sab']) + [p['pma']]):
        wq, bq = lin(sp['q'])
        wk, bk = lin(sp['k'])
        wv, bv = lin(sp['v'])
        wr1, br1 = lin(sp['r']['l1'])
        wr2, br2 = lin(sp['r']['l2'])
        tag = f's{i}' if i < 3 else 'pma'
        if i < 3:
            out[f'{tag}_wq'] = _bf(_pack(wq, 2))
            out[f'{tag}_i_bqv'] = add_bias(bq + bv)
        else:
            s = _np(p['S'])[0]                         # [4, 256]
            qp = s @ wq + bq + bv                      # [4, 256]
            out['qpmaT'] = _bf(_pack(qp.T, 2))         # [128, 8]
        out[f'{tag}_wk'] = _bf(_pack(wk, 2))
        out[f'{tag}_wv'] = _bf(_pack(wv, 2))
        out[f'{tag}_wr1'] = _pack(wr1, 2)
        out[f'{tag}_wr2'] = _bf(_pack(wr2, 2))
        out[f'{tag}_i_bk'] = add_bias(bk)
        out[f'{tag}_i_br1'] = add_bias(br1)
        out[f'{tag}_i_br2'] = add_bias(br2)
        add_ln(sp['g0'], sp['be0'])
        add_ln(sp['g1'], sp['be1'])

    w1c, b1c = lin(p['cls']['l1'])
    w2c, b2c = lin(p['cls']['l2'])
    out['w1c'] = _pack(w1c, 8)                         # [128, 2048]
    out['w2c'] = _pack(w2c, 2)                         # [128, 2]
    out['i_b1c'] = add_bias(b1c)
    out['b2c'] = float(b2c[0])

    # scalar constants needed as [128,1] bias APs on ScalarE
    import math as _m
    idx = len(bias_cols)
    bias_cols.append(np.full(128, HID * EPS, dtype=np.float32))
    out['i_ceps'] = idx
    idx = len(bias_cols)
    bias_cols.append(np.full(128, 0.5 * _m.log(float(HID)), dtype=np.float32))
    out['i_chl'] = idx

    out['bias_cols'] = np.stack(bias_cols, axis=1).astype(np.float32)  # [128, NB]

    # lnrows: [128, 8*512] bf16; LN l block at cols l*512, rows replicated at
    # partitions 0/32/64/96.  Within block: [g_c0 | g_c1 | b_c0 | b_c1].
    lnr = np.zeros((128, 8 * 512), dtype=np.float32)
    for l, blk in enumerate(ln_blocks):
        row = blk.reshape(-1)                          # [512]
        for pp in (0, 32, 64, 96):
            lnr[pp, l * 512:(l + 1) * 512] = row
    out['lnrows'] = _bf(lnr)

    out['iota'] = np.arange(128, dtype=np.float32).reshape(128, 1)
    out['ones'] = np.ones((128, 512), dtype=np.float32)
    out['ones_bf'] = np.ones((128, 512), dtype=ml_dtypes.bfloat16)
    return out


# --------------------------------------------------------------------------
# device program
# --------------------------------------------------------------------------

def _mm(nc, out, lhsT, rhs, **kw):
    nc.tensor.matmul(out, lhsT, rhs, **kw)


def build_nc(weights, debug=False):
    nc = bacc.Bacc("TRN2", target_bir_lowering=False, debug=False, num_devices=NC)

    dram = {}
    for k, v in weights.items():
        if isinstance(v, np.ndarray):
            dt = BF if v.dtype == ml_dtypes.bfloat16 else FP
            dram[k] = nc.dram_tensor(k, list(v.shape), dt, kind="ExternalInput")
    dram['xT'] = nc.dram_tensor('xT', [GPC, 32, NN], FP, kind="ExternalInput")
    dram['eaT'] = nc.dram_tensor('eaT', [GPC, 16, L], FP, kind="ExternalInput")
    dram['srcdst'] = nc.dram_tensor('srcdst', [GPC, 1, 2 * L], FP, kind="ExternalInput")
    out_dram = nc.dram_tensor('out', [1, GPC], FP, kind="ExternalOutput")
    dbg = {}
    if debug:
        for nm, ddt in (('e0', BF), ('x0', FP), ('o1', FP), ('e1', BF)):
            dbg[nm] = nc.dram_tensor(f'dbg_{nm}', [GPC, 2, 128, L], ddt,
                                     kind="ExternalOutput")
        dbg['pool'] = nc.dram_tensor('dbg_pool', [GPC, 2, 128, NI], FP,
                                     kind="ExternalOutput")
        dbg['rb'] = nc.dram_tensor('dbg_rb', [GPC, 2, 128, L], FP,
                                   kind="ExternalOutput")
        dbg['tmp'] = nc.dram_tensor('dbg_tmp', [GPC, 2, 128, L], FP,
                                    kind="ExternalOutput")
        dbg['qp'] = nc.dram_tensor('dbg_qp', [GPC, 2, 128, L], BF,
                                   kind="ExternalOutput")
        dbg['scp'] = nc.dram_tensor('dbg_scp', [2, 128, L], FP,
                                    kind="ExternalOutput")

    RS = 1.0 / math.sqrt(32.0)

    with tile.TileContext(nc) as tc:
        with (
            tc.tile_pool(name="wp", bufs=1) as wp,
            tc.tile_pool(name="act", bufs=1) as ap,
            tc.tile_pool(name="ps", bufs=1, space="PSUM") as pp,
        ):
            # ---- load constants & weights into SBUF ----
            W = {}
            for k, t in dram.items():
                if k in ('xT', 'eaT', 'srcdst', 'out'):
                    continue
                shape = list(t.shape)
                dt = t.dtype
                w_t = wp.tile(shape, dt, tag=k, name=f"w_{k}")
                nc.sync.dma_start(w_t[:], t.ap()[:])
                W[k] = w_t

            bias = W['bias_cols']

            def bcol(idx, c):
                return bias[:, idx + c:idx + c + 1]

            ones = W['ones']
            ones_bf = W['ones_bf']
            iota = W['iota']
            lnrows = W['lnrows']

            def wslice(w, k, m):
                # lhsT chunk [128, 128] of a packed [128, 2*256] weight
                return w[:, k * 256 + m * 128: k * 256 + (m + 1) * 128]

            E = [[None, None] for _ in range(GPC)]     # E^T chunks, bf16

            # ================= pre-phase: node/edge MLPs + eset =================
            for g in range(GPC):
                xT = ap.tile([32, NN], FP, tag="xT", bufs=2, name="xT")
                nc.sync.dma_start(xT[:], dram['xT'].ap()[g])
                eaT = ap.tile([16, L], FP, tag="eaT", bufs=1, name="eaT")
                nc.sync.dma_start(eaT[:], dram['eaT'].ap()[g])
                sd = ap.tile([1, 2 * L], FP, tag="sd", bufs=1, name="sd")
                nc.sync.dma_start(sd[:], dram['srcdst'].ap()[g])

                # node l1: h1T [256, 128] = Wn1.T @ xT (+bn1, relu)
                h1T = ap.tile([128, 2 * NN], FP, tag="h1T", bufs=2, name="h1T")
                for m in range(2):
                    ps = pp.tile([128, NN], FP, tag="acc", bufs=4, name="ps_n1")
                    _mm(nc, ps[:], W['wn1'][:, m * 128:(m + 1) * 128], xT[:])
                    nc.vector.tensor_scalar(
                        h1T[:, m * NN:(m + 1) * NN], ps[:],
                        bcol(weights['i_bn1'], m), 0.0, AluOpType.add, AluOpType.max)

                # U [128 nodes, 512] = relu1.T @ Wfused
                psu = pp.tile([128, L], FP, tag="acc", bufs=4, name="ps_u")
                for k in range(2):
                    _mm(nc, psu[:], h1T[:, k * NN:(k + 1) * NN],
                        W['wfused'][:, k * 512:(k + 1) * 512],
                        start=(k == 0), stop=(k == 1))
                U = ap.tile([128, L], FP, tag="U", bufs=2, name="U")
                nc.vector.tensor_scalar_add(U[:], psu[:], 0.0)

                # edge l1: r_eT [256, 512]
                reT = [None, None]
                for m in range(2):
                    ps = pp.tile([128, L], FP, tag="acc", bufs=4, name="ps_e1")
                    _mm(nc, ps[:], W['we1'][:, m * 128:(m + 1) * 128], eaT[:])
                    reT[m] = ap.tile([128, L], FP, tag="reT", bufs=2, name="reT")
                    nc.vector.tensor_scalar(
                        reT[m][:], ps[:],
                        bcol(weights['i_be1'], m), 0.0, AluOpType.add, AluOpType.max)

                # one-hot selectors
                sel = [None, None]
                for si in range(2):
                    psb = pp.tile([128, L], FP, tag="acc", bufs=4, name="ps_sel")
                    _mm(nc, psb[:], ones[0:1, 0:128], sd[:, si * L:(si + 1) * L])
                    sel[si] = ap.tile([128, L], FP, tag="sel", bufs=2, name="sel")
                    nc.vector.tensor_scalar(
                        sel[si][:], psb[:], iota[:, 0:1], None, AluOpType.is_equal)

                # eset l1 (fused): Y^T [256, 512]
                eh = [None, None]
                for c in range(2):
                    ps = pp.tile([128, L], FP, tag="acc", bufs=4, name="ps_y")
                    _mm(nc, ps[:], U[:, c * 128:(c + 1) * 128], sel[0][:],
                        start=True, stop=False)
                    _mm(nc, ps[:], U[:, 256 + c * 128:256 + (c + 1) * 128], sel[1][:],
                        start=False, stop=False)
                    for k in range(2):
                        _mm(nc, ps[:], wslice(W['wef'], k, c), reT[k][:],
                            start=False, stop=(k == 1))
                    eh[c] = ap.tile([128, L], FP, tag="eh", bufs=2, name="eh")
                    nc.vector.tensor_scalar(
                        eh[c][:], ps[:],
                        bcol(weights['i_b1p'], c), 0.0, AluOpType.add, AluOpType.max)

                # eset l2 -> E0 (bf16)
                for c in range(2):
                    ps = pp.tile([128, L], FP, tag="acc", bufs=4, name="ps_y2")
                    for k in range(2):
                        _mm(nc, ps[:], wslice(W['w2e'], k, c), eh[k][:],
                            start=(k == 0), stop=(k == 1))
                    et = ap.tile([128, L], BF, tag="E", bufs=12, name="E0")
                    nc.vector.tensor_scalar_add(et[:], ps[:], bcol(weights['i_b2e'], c))
                    E[g][c] = et
                    if debug:
                        nc.sync.dma_start(dbg['e0'].ap()[g, c], et[:])

            # ================= helper: batched layernorm =================
            def layernorm(ln_idx, X, out_tag, out_dtype, n, out_bufs=8, extra_emit=None):
                """X: [GPC][2] f32 tiles [128, n] -> returns new tiles (normalized).

                Emits col-tiled sums, stats, row-tiled (a, c) broadcasts and the
                apply.  extra_emit(g, tiles) is called after graph g's apply to
                interleave downstream per-graph work.
                """
                ssum = pp.tile([128, n], FP, tag="acc", bufs=4, name="ln_s")
                ssq = pp.tile([128, n], FP, tag="acc", bufs=4, name="ln_q")
                sqt = [[None, None] for _ in range(GPC)]
                for g in range(GPC):
                    for c in range(2):
                        sq = ap.tile([128, n], FP, tag="sq", bufs=2, name="sq")
                        nc.vector.tensor_mul(sq[:], X[g][c][:], X[g][c][:])
                        sqt[g][c] = sq
                    for c in range(2):
                        _mm(nc, ssum[32 * g:32 * g + 1, :], ones[:, 0:1], X[g][c][:],
                            start=(c == 0), stop=(c == 1),
                            tile_position=(0, 32 * g))
                    for c in range(2):
                        _mm(nc, ssq[32 * g:32 * g + 1, :], ones[:, 0:1], sqt[g][c][:],
                            start=(c == 0), stop=(c == 1),
                            tile_position=(0, 32 * g))
                # stats on rows {0,32,64,96}: process [97, n] contiguously
                P97 = 97
                scp = ap.tile([128, n], FP, tag="st", bufs=2, name="ln_scp")
                nc.vector.tensor_scalar_add(scp[0:P97, :], ssum[0:P97, :], 0.0)
                if debug and ln_idx == 0:
                    nc.sync.dma_start(dbg['scp'].ap()[0], scp[:])
                if debug and ln_idx == 1:
                    nc.sync.dma_start(dbg['scp'].ap()[1], scp[:])
                t1 = ap.tile([128, n], FP, tag="st", bufs=2, name="ln_t1")
                nc.vector.tensor_mul(t1[0:P97, :], scp[0:P97, :], scp[0:P97, :])
                t2 = ap.tile([128, n], FP, tag="st2", bufs=2, name="ln_t2")
                nc.vector.scalar_tensor_tensor(
                    t2[0:P97, :], t1[0:P97, :], -1.0 / HID, ssq[0:P97, :],
                    AluOpType.mult, AluOpType.add)      # t2 = ssq - ssum^2/256
                t3 = ap.tile([128, n], FP, tag="st2", bufs=2, name="ln_t3")
                nc.scalar.activation(t3[0:P97, :], t2[0:P97, :], AF.Ln,
                                     bias=bias[0:P97, weights['i_ceps']:weights['i_ceps'] + 1],
                                     scale=1.0)
                arow = ap.tile([128, n], BF, tag="sta", bufs=2, name="ln_a")
                nc.scalar.activation(arow[0:P97, :], t3[0:P97, :], AF.Exp,
                                     scale=-0.5,
                                     bias=bias[0:P97, weights['i_chl']:weights['i_chl'] + 1])
                crow = ap.tile([128, n], BF, tag="sta", bufs=2, name="ln_c")
                # c = -(sum/256) * rstd   (arow is bf16; DVE computes fp32 internally)
                t4 = ap.tile([128, n], FP, tag="st", bufs=2, name="ln_t4")
                nc.vector.tensor_mul(t4[0:P97, :], scp[0:P97, :], arow[0:P97, :])
                nc.vector.tensor_scalar_mul(crow[0:P97, :], t4[0:P97, :], -1.0 / HID)

                outs = [[None, None] for _ in range(GPC)]
                for g in range(GPC):
                    for c in range(2):
                        goff = ln_idx * 512 + c * 128          # gamma chunk c
                        boff = ln_idx * 512 + 256 + c * 128    # beta chunk c
                        ab = pp.tile([128, n], FP, tag="acc", bufs=4, name="ln_ab")
                        _mm(nc, ab[:], lnrows[32 * g:32 * g + 1, goff:goff + 128],
                            arow[32 * g:32 * g + 1, :], tile_position=(32 * g, 0))
                        cb = pp.tile([128, n], FP, tag="acc", bufs=4, name="ln_cb")
                        _mm(nc, cb[:], lnrows[32 * g:32 * g + 1, goff:goff + 128],
                            crow[32 * g:32 * g + 1, :],
                            start=True, stop=False, tile_position=(32 * g, 0))
                        _mm(nc, cb[:], lnrows[32 * g:32 * g + 1, boff:boff + 128],
                            ones_bf[32 * g:32 * g + 1, 0:n],
                            start=False, stop=True, tile_position=(32 * g, 0))
                        ot = ap.tile([128, n], out_dtype, tag=out_tag,
                                     bufs=out_bufs, name="ln_o")
                        tmp = ap.tile([128, n], FP, tag="tmp", bufs=2, name="ln_tmp")
                        nc.vector.tensor_mul(tmp[:], X[g][c][:], ab[:])
                        nc.vector.tensor_add(ot[:], tmp[:], cb[:])
                        outs[g][c] = ot
                    if extra_emit is not None:
                        extra_emit(g, outs[g])
                return outs

            # ================= SAB blocks =================
            def attention(tag, g, Ein, n, qpT, kpT, vp):
                """Emit scores/exp/denominator/attv; returns X tiles [2] (f32).

                qpT/kpT: [2] bf16 tiles [128, n]; vp: [4] bf16 tiles [128, 256]
                (node-major V, l-chunk each); Ein unused here.
                """
                Xt = [None, None]
                for hg in range(2):
                    exps = []
                    for mc in range(4):
                        sc = pp.tile([128, 4 * n], FP, tag="sc", bufs=1, name="sc")
                        for j in range(4):
                            _mm(nc, sc[:, j * n:(j + 1) * n],
                                kpT[hg][32 * j:32 * j + 32, mc * 128:(mc + 1) * 128],
                                qpT[hg][32 * j:32 * j + 32, :],
                                tile_position=(32 * j, 0))
                        ex = ap.tile([128, 4 * n], BF, tag="exps", bufs=4, name="exps")
                        nc.scalar.activation(ex[:], sc[:], AF.Exp, scale=RS)
                        exps.append(ex)
                    ou = pp.tile([128, n], FP, tag="acc", bufs=4, name="ou")
                    dn = pp.tile([128, n], FP, tag="acc", bufs=4, name="dn")
                    for mc in range(4):
                        for j in range(4):
                            _mm(nc, ou[32 * j:32 * j + 32, :],
                                vp[mc][:, (4 * hg + j) * 32:(4 * hg + j + 1) * 32],
                                exps[mc][:, j * n:(j + 1) * n],
                                start=(mc == 0),
                                stop=(mc == 3),
                                tile_position=(0, 32 * j))
                        for j in range(4):
                            _mm(nc, dn[32 * j:32 * j + 32, :],
                                ones_bf[:, 0:32],
                                exps[mc][:, j * n:(j + 1) * n],
                                start=(mc == 0),
                                stop=(mc == 3),
                                tile_position=(0, 32 * j))
                    lnd = ap.tile([128, n], FP, tag="recip", bufs=2, name="lnd")
                    nc.scalar.activation(lnd[:], dn[:], AF.Ln)
                    rb = ap.tile([128, n], FP, tag="recip", bufs=2, name="rb")
                    nc.scalar.activation(rb[:], lnd[:], AF.Exp, scale=-1.0)
                    tmp = ap.tile([128, n], FP, tag="tmp", bufs=2, name="att_t")
                    nc.vector.tensor_mul(tmp[:], ou[:], rb[:])
                    xt = ap.tile([128, n], FP, tag="X", bufs=8, name="X")
                    nc.vector.tensor_add(xt[:], tmp[:], qpT[hg][:])
                    Xt[hg] = xt
                    if debug and tag == 's0':
                        nc.sync.dma_start(dbg['x0'].ap()[g, hg], xt[:])
                        nc.sync.dma_start(dbg['rb'].ap()[g, hg], rb[:])
                        nc.sync.dma_start(dbg['tmp'].ap()[g, hg], tmp[:])
                        nc.sync.dma_start(dbg['qp'].ap()[g, hg], qpT[hg][:])
                return Xt

            def qkv(wq, wk, wv, i_bqv, i_bk, g, Ein, n, has_q=True):
                qpT = [None, None]
                kpT = [None, None]
                for c in range(2):
                    if has_q:
                        ps = pp.tile([128, n], FP, tag="acc", bufs=4, name="ps_q")
                        for k in range(2):
                            _mm(nc, ps[:], wslice(wq, k, c), Ein[k][:],
                                start=(k == 0), stop=(k == 1))
                        qt = ap.tile([128, n], BF, tag="qp", bufs=4, name="qpT")
                        nc.vector.tensor_scalar_add(qt[:], ps[:], bcol(i_bqv, c))
                        qpT[c] = qt
                    ps2 = pp.tile([128, n], FP, tag="acc", bufs=4, name="ps_k")
                    for k in range(2):
                        _mm(nc, ps2[:], wslice(wk, k, c), Ein[k][:],
                            start=(k == 0), stop=(k == 1))
                    kt = ap.tile([128, n], BF, tag="kp", bufs=4, name="kpT")
                    nc.vector.tensor_scalar_add(kt[:], ps2[:], bcol(i_bk, c))
                    kpT[c] = kt
                vp = []
                for mc in range(4):
                    ps3 = pp.tile([128, HID], FP, tag="acc", bufs=4, name="ps_v")
                    for k in range(2):
                        _mm(nc, ps3[:], Ein[k][:, mc * 128:(mc + 1) * 128],
                            wk_rhs(wv, k),
                            start=(k == 0), stop=(k == 1))
                    vt = ap.tile([128, HID], BF, tag="vp", bufs=8, name="vp")
                    nc.vector.tensor_scalar_add(vt[:], ps3[:], 0.0)
                    vp.append(vt)
                return qpT, kpT, vp

            def wk_rhs(w, k):
                return w[:, k * 256:(k + 1) * 256]

            def rmlp(wr1, wr2, i_br1, i_br2, g, O1, n):
                h1 = [None, None]
                for c in range(2):
                    ps = pp.tile([128, n], FP, tag="acc", bufs=4, name="ps_r1")
                    for k in range(2):
                        _mm(nc, ps[:], wslice(wr1, k, c), O1[k][:],
                            start=(k == 0), stop=(k == 1))
                    ht = ap.tile([128, n], BF, tag="h1r", bufs=4, name="h1r")
                    nc.vector.tensor_scalar(
                        ht[:], ps[:], bcol(i_br1, c), 0.0, AluOpType.add, AluOpType.max)
                    h1[c] = ht
                O2 = [None, None]
                for c in range(2):
                    ps = pp.tile([128, n], FP, tag="acc", bufs=4, name="ps_r2")
                    for k in range(2):
                        _mm(nc, ps[:], wslice(wr2, k, c), h1[k][:],
                            start=(k == 0), stop=(k == 1))
                    ot = ap.tile([128, n], FP, tag="O2", bufs=8, name="O2")
                    nc.vector.scalar_tensor_tensor(
                        ot[:], ps[:], bcol(i_br2, c), O1[c][:],
                        AluOpType.add, AluOpType.add)
                    O2[c] = ot
                return O2

            # bf16 rMLP inputs: O1 tiles are bf16, weights f32 -> need bf16 weights.
            # (wr1/wr2 left f32; O1 emitted f32 would double DVE cost... we pass
            # rhs bf16 against f32 lhsT is illegal; so keep rMLP fully f32: O1 f32.)

            for si in range(3):
                t = f's{si}'
                Xall = [None] * GPC
                for g in range(GPC):
                    qpT, kpT, vp = qkv(W[f'{t}_wq'], W[f'{t}_wk'], W[f'{t}_wv'],
                                       weights[f'{t}_i_bqv'], weights[f'{t}_i_bk'],
                                       g, E[g], L)
                    Xall[g] = attention(t, g, E[g], L, qpT, kpT, vp)

                O2all = [None] * GPC

                def after_ln1(g, O1, _si=si, _O2all=O2all, _t=t):
                    _O2all[g] = rmlp(W[f'{_t}_wr1'], W[f'{_t}_wr2'],
                                     weights[f'{_t}_i_br1'], weights[f'{_t}_i_br2'],
                                     g, O1, L)

                O1dump = layernorm(2 * si, Xall, "O1", FP, L, out_bufs=4,
                                   extra_emit=after_ln1)
                E = layernorm(2 * si + 1, O2all, "E", BF, L, out_bufs=12)
                if debug and si == 0:
                    for g in range(GPC):
                        for c in range(2):
                            nc.sync.dma_start(dbg['o1'].ap()[g, c], O1dump[g][c][:])
                            nc.sync.dma_start(dbg['e1'].ap()[g, c], E[g][c][:])

            # ================= PMA =================
            qpmaT = W['qpmaT']
            Xp = [None] * GPC
            for g in range(GPC):
                _, kpT, vp = qkv(None, W['pma_wk'], W['pma_wv'],
                                 None, weights['pma_i_bk'], g, E[g], L, has_q=False)
                # scores: s^T [m, l=4] per head; one 4-bank psum tile per hg
                Xt = [None, None]
                for hg in range(2):
                    sc = pp.tile([128, 2048], FP, tag="sc", bufs=1, name="sc_p")
                    for mc in range(4):
                        for j in range(4):
                            _mm(nc, sc[:, j * 512 + mc * NI: j * 512 + (mc + 1) * NI],
                                kpT[hg][32 * j:32 * j + 32, mc * 128:(mc + 1) * 128],
                                qpmaT[32 * j:32 * j + 32, hg * NI:(hg + 1) * NI],
                                start=True, stop=True,
                                tile_position=(32 * j, 0))
                    scv = sc.rearrange("p (h c) -> p h c", h=4)[:, :, 0:4 * NI]
                    ex = ap.tile([128, 4, 4 * NI], BF, tag="exps_p", bufs=2, name="exps_p")
                    nc.scalar.activation(ex[:], scv, AF.Exp, scale=RS)
                    ou = pp.tile([128, NI], FP, tag="acc", bufs=4, name="ou_p")
                    dn = pp.tile([128, NI], FP, tag="acc", bufs=4, name="dn_p")
                    for mc in range(4):
                        for j in range(4):
                            _mm(nc, ou[32 * j:32 * j + 32, :],
                                vp[mc][:, (4 * hg + j) * 32:(4 * hg + j + 1) * 32],
                                ex[:, j, mc * NI:(mc + 1) * NI],
                                start=(mc == 0), stop=(mc == 3),
                                tile_position=(0, 32 * j))
                        for j in range(4):
                            _mm(nc, dn[32 * j:32 * j + 32, :],
                                ones_bf[:, 0:32],
                                ex[:, j, mc * NI:(mc + 1) * NI],
                                start=(mc == 0), stop=(mc == 3),
                                tile_position=(0, 32 * j))
                    lnd = ap.tile([128, NI], FP, tag="recip_p", bufs=2, name="lnd_p")
                    nc.scalar.activation(lnd[:], dn[:], AF.Ln)
                    rb = ap.tile([128, NI], FP, tag="recip_p", bufs=2, name="rb_p")
                    nc.scalar.activation(rb[:], lnd[:], AF.Exp, scale=-1.0)
                    tmp = ap.tile([128, NI], FP, tag="tmp_p", bufs=2, name="t_p")
                    nc.vector.tensor_mul(tmp[:], ou[:], rb[:])
                    xt = ap.tile([128, NI], FP, tag="Xp", bufs=8, name="Xp")
                    nc.vector.tensor_add(xt[:], tmp[:], qpmaT[:, hg * NI:(hg + 1) * NI])
                    Xt[hg] = xt
                Xp[g] = Xt

            O2p = [None] * GPC

            def after_ln1p(g, O1):
                O2p[g] = rmlp(W['pma_wr1'], W['pma_wr2'],
                              weights['pma_i_br1'], weights['pma_i_br2'], g, O1, NI)

            layernorm(6, Xp, "O1p", FP, NI, out_bufs=4, extra_emit=after_ln1p)
            pooled = layernorm(7, O2p, "pooled", FP, NI, out_bufs=8)
            if debug:
                for g in range(GPC):
                    for c in range(2):
                        nc.sync.dma_start(dbg['pool'].ap()[g, c], pooled[g][c][:])

            # ================= classifier =================
            outsb = ap.tile([1, GPC], FP, tag="outsb", bufs=1, name="outsb")
            for g in range(GPC):
                h1c = [None, None]
                for m in range(2):
                    ps = pp.tile([128, 1], FP, tag="acc", bufs=4, name="ps_c1")
                    for kk in range(8):
                        ll, cc = kk // 2, kk % 2
                        _mm(nc, ps[:],
                            W['w1c'][:, kk * 256 + m * 128: kk * 256 + (m + 1) * 128],
                            pooled[g][cc][:, ll:ll + 1],
                            start=(kk == 0), stop=(kk == 7))
                    ht = ap.tile([128, 1], FP, tag="h1c", bufs=4, name="h1c")
                    nc.vector.tensor_scalar(
                        ht[:], ps[:], bcol(weights['i_b1c'], m), 0.0,
                        AluOpType.add, AluOpType.max)
                    h1c[m] = ht
                ps2 = pp.tile([1, 1], FP, tag="acc", bufs=4, name="ps_c2")
                for m in range(2):
                    _mm(nc, ps2[:], W['w2c'][:, m:m + 1], h1c[m][:],
                        start=(m == 0), stop=(m == 1))
                nc.vector.tensor_scalar_add(outsb[0:1, g:g + 1], ps2[:],
                                            weights['b2c'])
            nc.sync.dma_start(out_dram.ap()[:], outsb[:])

    nc.finalize()
    return nc


# --------------------------------------------------------------------------
# entry point
# --------------------------------------------------------------------------

def kernel(x, edge_index, edge_attr, params):
    x = np.asarray(x, dtype=np.float32)
    edge_index = np.asarray(edge_index)
    edge_attr = np.asarray(edge_attr, dtype=np.float32)

    weights = prep_weights(params)
    key = 'nc'
    if key not in _CACHE:
        _CACHE[key] = build_nc(weights)
    nc = _CACHE[key]

    B = x.shape[0]
    in_maps = []
    for c in range(NC):
        sl = slice(c * GPC, (c + 1) * GPC)
        m = {k: v for k, v in weights.items() if isinstance(v, np.ndarray)}
        m['xT'] = np.ascontiguousarray(x[sl].transpose(0, 2, 1))
        m['eaT'] = np.ascontiguousarray(edge_attr[sl].transpose(0, 2, 1))
        ei = edge_index[sl].astype(np.float32)          # [4, 2, 512]
        m['srcdst'] = np.ascontiguousarray(ei.reshape(GPC, 1, 2 * L))
        in_maps.append(m)

    res = bass_utils.run_bass_kernel_spmd(nc, in_maps, core_ids=list(range(NC)))
    logits = np.concatenate([r['out'].reshape(-1) for r in res.results])
    return logits.astype(np.float32)
